# revision 33
# baseline (speedup 1.0000x reference)
"""Mixed-precision quantized linear (fp32/int8/int4/int2 weight groups) on 8 trn2 cores.

Strategy v7: tensor-parallel over output channels. Core k owns rows
[k*n_g/8, (k+1)*n_g/8) of every bit-group (128 + 384 + 512 + 256 = 1280
channels). x replicated. K globally permuted evens-then-odds (legacy of the
packed-nibble layout; kept so all staged operands agree).

Per-group engine/dtype plan (measured rel err ~1.2% < 2e-2 gate):
- w16 + q8 (512 ch): one 512-wide bf16 matmul per (K tile, token block)
  (int8 exact in bf16), weights shipped bf16.
- p4 (512 ch): host-unpacked to fp8e4m3 (ints [-8,7] exact), fp8 DoubleRow
  matmuls (2 K-tiles per pass, 2x PE throughput), x shipped as fp8 too.
- p2 (256 ch): host-unpacked to int8, cast int8->bf16 on DVE (Pool/ACT
  int8 casts are ~15x slower on HW than modeled), matmul'd in bf16
  (2-bit group carries 54% of output energy; fp8 x there fails the gate).

All three groups interleave per K-tile pair on the PE so DMA feeds at a
steady rate and consecutive matmuls share stationary tiles. Bias row and
scales DMA first (the K=1 bias matmuls must not stall the PE queue).
Warmup matmuls ramp the PE p-state during DMA spin-up (an idle PE drops
back to 1.2 GHz). Epilogue (psum+bias)*scale on DVE writes bf16; host
upcasts and scatters the 8 core slices into [256, 11008].
"""

import numpy as np
import ml_dtypes

import concourse.bass as bass
import concourse.bacc as bacc
import concourse.mybir as mybir
import concourse.tile as tile
from concourse.bass_utils import run_bass_kernel_spmd

IN = 4096
OUT = 11008
N16, N8, N4, N2 = 1024, 3072, 4096, 2048
M = 256
NCORES = 8
C16, C8, C4, C2 = N16 // 8, N8 // 8, N4 // 8, N2 // 8  # 128, 384, 512, 256
NCH = C16 + C8 + C4 + C2  # 1280
KT = IN // 128  # 32 K-tiles
KP = KT // 2  # 16 DoubleRow pairs
CWQ = C16 + C8  # 512 bf16 "wq" group = [w16 | q8]

BF16 = mybir.dt.bfloat16
F32 = mybir.dt.float32
I8 = mybir.dt.int8
F8 = mybir.dt.float8e4

Alu = mybir.AluOpType
DRM = mybir.MatmulPerfMode.DoubleRow

WARM_MM = 9  # p-state ramp matmuls issued before real work


def _enable_ldw_opt():
    """Turn on the compiler's ldweights optimization for this NEFF: the
    matmul stream is ordered so consecutive matmuls share stationary
    tiles, which the (default-off) ldw-opt pass can exploit."""
    try:
        from concourse.compiler_utils import get_compiler_flags, set_compiler_flags

        flags = [
            f.replace("--enable-ldw-opt=false", "--enable-ldw-opt=true")
            for f in get_compiler_flags()
        ]
        set_compiler_flags(flags)
    except Exception:
        pass


def _build_nc():
    nc = bacc.Bacc()
    xt_d = nc.declare_dram_parameter("xt", [128, KT * M], BF16, isOutput=False)
    x8_d = nc.declare_dram_parameter("x8", [128, KT * M], F8, isOutput=False)
    wq_d = nc.declare_dram_parameter("wq", [128, KT * CWQ], BF16, isOutput=False)
    p4_d = nc.declare_dram_parameter("p4u", [128, KT * C4], F8, isOutput=False)
    p2_d = nc.declare_dram_parameter("p2u", [128, KT * C2], I8, isOutput=False)
    sbc_d = nc.declare_dram_parameter("sbc", [128, NCH], BF16, isOutput=False)
    brow_d = nc.declare_dram_parameter("brow", [1, NCH], BF16, isOutput=False)
    out_d = nc.declare_dram_parameter("out", [M, NCH], BF16, isOutput=True)

    with tile.TileContext(nc) as tc:
        with (
            tc.tile_pool(name="big", bufs=1) as pool,
            tc.tile_pool(name="psum", bufs=1, space="PSUM") as ppool,
        ):
            xs = pool.tile([128, KT, 2, 128], BF16)  # [p, kt, blk, tok]
            x8s = pool.tile([128, KT, 2, 128], F8)
            wqs = pool.tile([128, KT, CWQ], BF16)  # [p, kt, ch]
            p4s = pool.tile([128, KT, C4], F8)
            p2s = pool.tile([128, KT, C2], I8)
            p2d = pool.tile([128, KT, C2], BF16)
            sbcs = pool.tile([128, NCH], BF16)
            brs = pool.tile([1, NCH], BF16)
            brs2 = pool.tile([1, NCH], BF16)
            ones_b = pool.tile([1, 128], BF16)
            outs = pool.tile([128, 2, NCH], BF16)  # [p, blk, ch]
            warm = pool.tile([128, 512], BF16)

            # ---- input DMAs: bias/scales first (tiny), then the operand
            # streams in consumption order, round-robin across the two
            # trigger queues (sync=SP, scalar=ACT). x8 is derived on-device.
            nc.scalar.dma_start(out=brs[:], in_=brow_d[:])
            nc.sync.dma_start(out=sbcs[:], in_=sbc_d[:])
            engs = [nc.sync, nc.scalar]
            ei = 0

            def dma(out, in_):
                nonlocal ei
                engs[ei % 2].dma_start(out=out, in_=in_)
                ei += 1

            for k0, k1 in ((0, 4), (4, 8)):
                dma(wqs[:, k0:k1], wq_d[:, k0 * CWQ : k1 * CWQ])
                dma(xs[:, k0:k1], xt_d[:, k0 * 256 : k1 * 256])
                dma(p2s[:, k0:k1], p2_d[:, k0 * C2 : k1 * C2])
                dma(x8s[:, k0:k1], x8_d[:, k0 * 256 : k1 * 256])
                dma(p4s[:, k0:k1], p4_d[:, k0 * C4 : k1 * C4])
            for g in range(1, 4):
                k0, k1 = g * 8, g * 8 + 8
                dma(wqs[:, k0:k1], wq_d[:, k0 * CWQ : k1 * CWQ])
                dma(xs[:, k0:k1], xt_d[:, k0 * 256 : k1 * 256])
                dma(x8s[:, k0:k1], x8_d[:, k0 * 256 : k1 * 256])
                dma(p4s[:, k0:k1], p4_d[:, k0 * C4 : k1 * C4])
                dma(p2s[:, k0:k1], p2_d[:, k0 * C2 : k1 * C2])

            # ---- PE p-state ramp while DMAs stream (memset on Pool: its
            # preamble ends earliest, so warmup starts sooner)
            nc.gpsimd.memset(warm[:], 1.0)
            warm_ps = ppool.tile([128, 512], F32, name="warm", tag="warm")
            for _ in range(WARM_MM):
                nc.tensor.matmul(
                    warm_ps[:], warm[:, :128], warm[:],
                    start=True, stop=True, skip_group_check=True,
                )

            nc.vector.memset(ones_b[:], 1.0)
            # bounce the bias row through DVE so the K=1 bias matmuls
            # have all-DVE deps (matmul carries only one sem wait)
            nc.vector.tensor_copy(brs2[:], brs[:])

            # ---- p2 int8 -> bf16 casts on DVE (the one fast cast path on
            # this HW: fp8 outputs and Pool/ACT int8 casts are ~15x slower
            # than modeled), chunked to chase the p2u DMA chunks
            for k0, k1 in ((0, 4), (4, 8), (8, 16), (16, 24), (24, 32)):
                nc.vector.tensor_scalar(
                    p2d[:, k0:k1], p2s[:, k0:k1], 1.0, None, op0=Alu.mult,
                )

            # ---- psums (2KB banks)
            ps_wq = [
                ppool.tile([128, CWQ], F32, name=f"wq{b}", tag=f"wq{b}")
                for b in range(2)
            ]
            ps_f8 = [
                ppool.tile([128, C4], F32, name=f"f8{b}", tag=f"f8{b}")
                for b in range(2)
            ]
            ps_p2 = ppool.tile([128, 2 * C2], F32, name="p2", tag="p2")

            def mm_wq(kt, b):
                nc.tensor.matmul(
                    ps_wq[b][:], xs[:, kt, b], wqs[:, kt],
                    start=(kt == 0), stop=False, skip_group_check=True,
                )

            def mm_p2(kt, b):
                nc.tensor.matmul(
                    ps_p2[:, b * C2 : b * C2 + C2], xs[:, kt, b], p2d[:, kt],
                    start=(kt == 0 and b == 0), stop=False,
                    skip_group_check=True,
                )

            def mm_f8(i, b, h):
                nc.tensor.matmul(
                    ps_f8[b][:, h * 256 : h * 256 + 256],
                    x8s[:, 2 * i : 2 * i + 2, b],
                    p4s[:, 2 * i : 2 * i + 2, h * 256 : h * 256 + 256],
                    start=(i == 0 and h == 0), stop=False,
                    perf_mode=DRM, skip_group_check=True,
                )

            def epilogue(ps, psc0, c0, cw, b):
                nc.tensor.matmul(
                    ps[:, psc0 : psc0 + cw], ones_b[:1, :], brs2[:1, c0 : c0 + cw],
                    start=False, stop=True, skip_group_check=True,
                )
                nc.vector.scalar_tensor_tensor(
                    outs[:, b, c0 : c0 + cw],
                    ps[:, psc0 : psc0 + cw], 1.0, sbcs[:, c0 : c0 + cw],
                    op0=Alu.mult, op1=Alu.mult,
                )

            # ---- main stream: groups interleaved per K-tile pair for the
            # first 24 kts (steady DMA-rate consumption, shared stationary
            # tiles); then staggered endings so each group's bias matmul +
            # DVE epilogue + output DMA overlap the next group's matmuls.
            for i in range(12):
                for kt in (2 * i, 2 * i + 1):
                    for b in range(2):
                        mm_wq(kt, b)
                        mm_p2(kt, b)
                for b in range(2):
                    for h in range(2):
                        mm_f8(i, b, h)

            out_v = out_d[:].rearrange("(b p) n -> p b n", p=128)
            for kt in range(24, 32):
                for b in range(2):
                    mm_wq(kt, b)
            for b in range(2):
                epilogue(ps_wq[b], 0, 0, CWQ, b)
            for i in range(12, 16):
                for b in range(2):
                    for h in range(2):
                        mm_f8(i, b, h)
            nc.sync.dma_start(out=out_v[:, 0, 0:512], in_=outs[:, 0, 0:512])
            nc.scalar.dma_start(out=out_v[:, 1, 0:512], in_=outs[:, 1, 0:512])
            for b in range(2):
                epilogue(ps_f8[b], 0, CWQ, C4, b)
            for kt in range(24, 32):
                for b in range(2):
                    mm_p2(kt, b)
            nc.sync.dma_start(out=out_v[:, 0, 512:1024], in_=outs[:, 0, 512:1024])
            nc.scalar.dma_start(out=out_v[:, 1, 512:1024], in_=outs[:, 1, 512:1024])
            for b in range(2):
                epilogue(ps_p2, b * C2, 1024, C2, b)
            nc.sync.dma_start(out=out_v[:, 0, 1024:], in_=outs[:, 0, 1024:])
            nc.scalar.dma_start(out=out_v[:, 1, 1024:], in_=outs[:, 1, 1024:])
    nc.finalize()
    return nc


def _tile128(a):
    """[K, F] -> [128, (K//128)*F] so DRAM layout matches the SBUF tile."""
    k, f = a.shape
    t = k // 128
    return np.ascontiguousarray(
        a.reshape(t, 128, f).transpose(1, 0, 2).reshape(128, t * f)
    )


def _unpack_nibbles(p):
    """packed int8 [N, K//2] -> int [N, K] (low nibble = even k)."""
    u = p.view(np.uint8) if p.dtype == np.int8 else p.astype(np.uint8)
    lo = (u & 15).astype(np.int16)
    hi = ((u >> 4) & 15).astype(np.int16)
    full = np.stack([lo, hi], axis=-1).reshape(p.shape[0], -1)
    return np.where(full > 7, full - 16, full)


_CACHE = {}


def stage_inputs(**inputs):
    x = np.asarray(inputs["x"], dtype=np.float32)
    w16 = np.asarray(inputs["w16"], dtype=np.float32)
    b16 = np.asarray(inputs["b16"], dtype=np.float32)
    q8 = np.asarray(inputs["q8"])
    s8 = np.asarray(inputs["s8"], dtype=np.float32)
    b8 = np.asarray(inputs["b8"], dtype=np.float32)
    p4 = np.asarray(inputs["p4"]).astype(np.int8)
    s4 = np.asarray(inputs["s4"], dtype=np.float32)
    b4 = np.asarray(inputs["b4"], dtype=np.float32)
    p2 = np.asarray(inputs["p2"]).astype(np.int8)
    s2 = np.asarray(inputs["s2"], dtype=np.float32)
    b2 = np.asarray(inputs["b2"], dtype=np.float32)
    idx16 = np.asarray(inputs["idx16"])
    idx8 = np.asarray(inputs["idx8"])
    idx4 = np.asarray(inputs["idx4"])
    idx2 = np.asarray(inputs["idx2"])

    bf16 = ml_dtypes.bfloat16
    f8 = ml_dtypes.float8_e4m3
    permK = np.concatenate([np.arange(0, IN, 2), np.arange(1, IN, 2)])

    # x: [p, kt, blk, tok] flattened; bf16 and fp8 copies
    xTp = np.ascontiguousarray(x.T[permK])  # [4096, 256] f32
    xt4 = xTp.reshape(KT, 128, 2, 128).transpose(1, 0, 2, 3)  # [p, kt, blk, tok]
    xt = np.ascontiguousarray(xt4.reshape(128, KT * M)).astype(bf16)
    x8 = np.ascontiguousarray(xt4.reshape(128, KT * M)).astype(f8)

    q4_full = _unpack_nibbles(p4)  # [N4, 4096] ints
    q2_full = _unpack_nibbles(p2)  # [N2, 4096] ints

    in_maps = []
    for k in range(NCORES):
        w16k = w16[k * C16 : (k + 1) * C16]
        q8k = q8[k * C8 : (k + 1) * C8]
        q4k = q4_full[k * C4 : (k + 1) * C4]
        q2k = q2_full[k * C2 : (k + 1) * C2]
        s8k = s8[k * C8 : (k + 1) * C8, 0]
        s4k = s4[k * C4 : (k + 1) * C4, 0]
        s2k = s2[k * C2 : (k + 1) * C2, 0]
        b16k = b16[k * C16 : (k + 1) * C16]
        b8k = b8[k * C8 : (k + 1) * C8]
        b4k = b4[k * C4 : (k + 1) * C4]
        b2k = b2[k * C2 : (k + 1) * C2]

        wq = _tile128(
            np.ascontiguousarray(
                np.concatenate([w16k.T, q8k.astype(np.float32).T], axis=1)[permK]
            ).astype(bf16)
        )
        p4u = _tile128(
            np.ascontiguousarray(q4k.astype(np.float32).T[permK]).astype(f8)
        )
        p2u = _tile128(np.ascontiguousarray(q2k.astype(np.int8).T[permK]))

        srow = np.concatenate([np.ones(C16, np.float32), s8k, s4k, s2k])
        sbc = np.ascontiguousarray(
            np.broadcast_to(srow[None, :], (128, NCH))
        ).astype(bf16)
        brow = (
            np.concatenate([b16k, b8k / s8k, b4k / s4k, b2k / s2k])
            .reshape(1, NCH)
            .astype(bf16)
        )

        in_maps.append(
            {"xt": xt, "x8": x8, "wq": wq, "p4u": p4u, "p2u": p2u,
             "sbc": sbc, "brow": brow}
        )

    cat_idxs = [
        np.concatenate(
            [
                idx16[k * C16 : (k + 1) * C16],
                idx8[k * C8 : (k + 1) * C8],
                idx4[k * C4 : (k + 1) * C4],
                idx2[k * C2 : (k + 1) * C2],
            ]
        )
        for k in range(NCORES)
    ]
    return in_maps, cat_idxs


def kernel(**inputs):
    in_maps, cat_idxs = stage_inputs(**inputs)
    if "nc" not in _CACHE:
        _enable_ldw_opt()
        _CACHE["nc"] = _build_nc()
    res = run_bass_kernel_spmd(_CACHE["nc"], in_maps, core_ids=list(range(NCORES)))
    _CACHE["last_res"] = res

    out = np.zeros((M, OUT), dtype=np.float32)
    for k in range(NCORES):
        out[:, cat_idxs[k]] = res.results[k]["out"].astype(np.float32)
    return out


# revision 34
# speedup vs baseline: 1.1002x; 1.1002x over previous
"""Mixed-precision quantized linear (fp32/int8/int4/int2 weight groups) on 8 trn2 cores.

Strategy v7: tensor-parallel over output channels. Core k owns rows
[k*n_g/8, (k+1)*n_g/8) of every bit-group (128 + 384 + 512 + 256 = 1280
channels). x replicated. K globally permuted evens-then-odds (legacy of the
packed-nibble layout; kept so all staged operands agree).

Per-group engine/dtype plan (measured rel err ~1.2% < 2e-2 gate):
- w16 + q8 (512 ch): one 512-wide bf16 matmul per (K tile, token block)
  (int8 exact in bf16), weights shipped bf16.
- p4 (512 ch): host-unpacked to fp8e4m3 (ints [-8,7] exact), fp8 DoubleRow
  matmuls (2 K-tiles per pass, 2x PE throughput), x shipped as fp8 too.
- p2 (256 ch): host-unpacked to int8, cast int8->bf16 on DVE (Pool/ACT
  int8 casts are ~15x slower on HW than modeled), matmul'd in bf16
  (2-bit group carries 54% of output energy; fp8 x there fails the gate).

All three groups interleave per K-tile pair on the PE so DMA feeds at a
steady rate and consecutive matmuls share stationary tiles. Bias row and
scales DMA first (the K=1 bias matmuls must not stall the PE queue).
Warmup matmuls ramp the PE p-state during DMA spin-up (an idle PE drops
back to 1.2 GHz). Epilogue (psum+bias)*scale on DVE writes bf16; host
upcasts and scatters the 8 core slices into [256, 11008].
"""

import numpy as np
import ml_dtypes

import concourse.bass as bass
import concourse.bacc as bacc
import concourse.mybir as mybir
import concourse.tile as tile
from concourse.bass_utils import run_bass_kernel_spmd

IN = 4096
OUT = 11008
N16, N8, N4, N2 = 1024, 3072, 4096, 2048
M = 256
NCORES = 8
C16, C8, C4, C2 = N16 // 8, N8 // 8, N4 // 8, N2 // 8  # 128, 384, 512, 256
NCH = C16 + C8 + C4 + C2  # 1280
KT = IN // 128  # 32 K-tiles
KP = KT // 2  # 16 DoubleRow pairs
CWQ = C16 + C8  # 512 bf16 "wq" group = [w16 | q8]

BF16 = mybir.dt.bfloat16
F32 = mybir.dt.float32
I8 = mybir.dt.int8
F8 = mybir.dt.float8e4

Alu = mybir.AluOpType
DRM = mybir.MatmulPerfMode.DoubleRow

WARM_MM = 9  # p-state ramp matmuls issued before real work


def _enable_ldw_opt():
    """Turn on the compiler's ldweights optimization for this NEFF: the
    matmul stream is ordered so consecutive matmuls share stationary
    tiles, which the (default-off) ldw-opt pass can exploit."""
    try:
        from concourse.compiler_utils import get_compiler_flags, set_compiler_flags

        flags = [
            f.replace("--enable-ldw-opt=false", "--enable-ldw-opt=true")
            for f in get_compiler_flags()
        ]
        set_compiler_flags(flags)
    except Exception:
        pass


def _build_nc():
    nc = bacc.Bacc()
    xt_d = nc.declare_dram_parameter("xt", [128, KT * M], BF16, isOutput=False)
    x8_d = nc.declare_dram_parameter("x8", [128, KT * M], F8, isOutput=False)
    w16_d = nc.declare_dram_parameter("w16t", [128, KT * C16], BF16, isOutput=False)
    q8_d = nc.declare_dram_parameter("q8t", [128, KT * C8], I8, isOutput=False)
    p4_d = nc.declare_dram_parameter("p4u", [128, KT * C4], F8, isOutput=False)
    p2_d = nc.declare_dram_parameter("p2u", [128, KT * C2], I8, isOutput=False)
    sbc_d = nc.declare_dram_parameter("sbc", [128, NCH], BF16, isOutput=False)
    brow_d = nc.declare_dram_parameter("brow", [1, NCH], BF16, isOutput=False)
    out_d = nc.declare_dram_parameter("out", [M, NCH], BF16, isOutput=True)

    with tile.TileContext(nc) as tc:
        with (
            tc.tile_pool(name="big", bufs=1) as pool,
            tc.tile_pool(name="psum", bufs=1, space="PSUM") as ppool,
        ):
            xs = pool.tile([128, KT, 2, 128], BF16)  # [p, kt, blk, tok]
            x8s = pool.tile([128, KT, 2, 128], F8)
            wqs = pool.tile([128, KT, CWQ], BF16)  # [p, kt, ch]
            q8s = pool.tile([128, KT, C8], I8)
            p4s = pool.tile([128, KT, C4], F8)
            p2s = pool.tile([128, KT, C2], I8)
            p2d = pool.tile([128, KT, C2], BF16)
            sbcs = pool.tile([128, NCH], BF16)
            brs = pool.tile([1, NCH], BF16)
            brs2 = pool.tile([1, NCH], BF16)
            ones_b = pool.tile([1, 128], BF16)
            outs = pool.tile([128, 2, NCH], BF16)  # [p, blk, ch]
            warm = pool.tile([128, 512], BF16)

            # ---- input DMAs: bias/scales first (tiny), then the operand
            # streams in consumption order, round-robin across the two
            # trigger queues (sync=SP, scalar=ACT). x8 is derived on-device.
            nc.scalar.dma_start(out=brs[:], in_=brow_d[:])
            nc.sync.dma_start(out=sbcs[:], in_=sbc_d[:])
            engs = [nc.sync, nc.scalar]
            ei = 0

            def dma(out, in_):
                nonlocal ei
                engs[ei % 2].dma_start(out=out, in_=in_)
                ei += 1

            for k0, k1 in ((0, 4), (4, 8)):
                dma(q8s[:, k0:k1], q8_d[:, k0 * C8 : k1 * C8])
                dma(wqs[:, k0:k1, 0:C16], w16_d[:, k0 * C16 : k1 * C16])
                dma(xs[:, k0:k1], xt_d[:, k0 * 256 : k1 * 256])
                dma(p2s[:, k0:k1], p2_d[:, k0 * C2 : k1 * C2])
                dma(x8s[:, k0:k1], x8_d[:, k0 * 256 : k1 * 256])
                dma(p4s[:, k0:k1], p4_d[:, k0 * C4 : k1 * C4])
            for g in range(1, 4):
                k0, k1 = g * 8, g * 8 + 8
                dma(q8s[:, k0:k1], q8_d[:, k0 * C8 : k1 * C8])
                dma(wqs[:, k0:k1, 0:C16], w16_d[:, k0 * C16 : k1 * C16])
                dma(xs[:, k0:k1], xt_d[:, k0 * 256 : k1 * 256])
                dma(x8s[:, k0:k1], x8_d[:, k0 * 256 : k1 * 256])
                dma(p4s[:, k0:k1], p4_d[:, k0 * C4 : k1 * C4])
                dma(p2s[:, k0:k1], p2_d[:, k0 * C2 : k1 * C2])

            # ---- PE p-state ramp while DMAs stream (memset on Pool: its
            # preamble ends earliest, so warmup starts sooner)
            nc.gpsimd.memset(warm[:], 1.0)
            warm_ps = ppool.tile([128, 512], F32, name="warm", tag="warm")
            for _ in range(WARM_MM):
                nc.tensor.matmul(
                    warm_ps[:], warm[:, :128], warm[:],
                    start=True, stop=True, skip_group_check=True,
                )

            nc.vector.memset(ones_b[:], 1.0)
            # bounce the bias row through DVE so the K=1 bias matmuls
            # have all-DVE deps (matmul carries only one sem wait)
            nc.vector.tensor_copy(brs2[:], brs[:])

            # ---- q8 and p2 int8 -> bf16 casts on DVE (the one fast cast
            # path on this HW: fp8 outputs and Pool/ACT int8 casts are
            # ~15x slower than modeled), chunked to chase the DMA chunks
            for k0, k1 in ((0, 4), (4, 8), (8, 16), (16, 24), (24, 32)):
                nc.vector.tensor_scalar(
                    wqs[:, k0:k1, C16:CWQ], q8s[:, k0:k1],
                    1.0, None, op0=Alu.mult,
                )
                nc.vector.tensor_scalar(
                    p2d[:, k0:k1], p2s[:, k0:k1], 1.0, None, op0=Alu.mult,
                )

            # ---- psums (2KB banks)
            ps_wq = [
                ppool.tile([128, CWQ], F32, name=f"wq{b}", tag=f"wq{b}")
                for b in range(2)
            ]
            ps_f8 = [
                ppool.tile([128, C4], F32, name=f"f8{b}", tag=f"f8{b}")
                for b in range(2)
            ]
            ps_p2 = ppool.tile([128, 2 * C2], F32, name="p2", tag="p2")

            def mm_wq(kt, b):
                nc.tensor.matmul(
                    ps_wq[b][:], xs[:, kt, b], wqs[:, kt],
                    start=(kt == 0), stop=False, skip_group_check=True,
                )

            def mm_p2(kt, b):
                nc.tensor.matmul(
                    ps_p2[:, b * C2 : b * C2 + C2], xs[:, kt, b], p2d[:, kt],
                    start=(kt == 0 and b == 0), stop=False,
                    skip_group_check=True,
                )

            def mm_f8(i, b, h):
                nc.tensor.matmul(
                    ps_f8[b][:, h * 256 : h * 256 + 256],
                    x8s[:, 2 * i : 2 * i + 2, b],
                    p4s[:, 2 * i : 2 * i + 2, h * 256 : h * 256 + 256],
                    start=(i == 0 and h == 0), stop=False,
                    perf_mode=DRM, skip_group_check=True,
                )

            def epilogue(ps, psc0, c0, cw, b):
                nc.tensor.matmul(
                    ps[:, psc0 : psc0 + cw], ones_b[:1, :], brs2[:1, c0 : c0 + cw],
                    start=False, stop=True, skip_group_check=True,
                )
                nc.vector.scalar_tensor_tensor(
                    outs[:, b, c0 : c0 + cw],
                    ps[:, psc0 : psc0 + cw], 1.0, sbcs[:, c0 : c0 + cw],
                    op0=Alu.mult, op1=Alu.mult,
                )

            # ---- main stream: groups interleaved per K-tile pair for the
            # first 24 kts (steady DMA-rate consumption, shared stationary
            # tiles); then staggered endings so each group's bias matmul +
            # DVE epilogue + output DMA overlap the next group's matmuls.
            for i in range(12):
                for kt in (2 * i, 2 * i + 1):
                    for b in range(2):
                        mm_wq(kt, b)
                        mm_p2(kt, b)
                for b in range(2):
                    for h in range(2):
                        mm_f8(i, b, h)

            out_v = out_d[:].rearrange("(b p) n -> p b n", p=128)
            for kt in range(24, 32):
                for b in range(2):
                    mm_wq(kt, b)
            for b in range(2):
                epilogue(ps_wq[b], 0, 0, CWQ, b)
            for i in range(12, 16):
                for b in range(2):
                    for h in range(2):
                        mm_f8(i, b, h)
            nc.sync.dma_start(out=out_v[:, 0, 0:512], in_=outs[:, 0, 0:512])
            nc.scalar.dma_start(out=out_v[:, 1, 0:512], in_=outs[:, 1, 0:512])
            for b in range(2):
                epilogue(ps_f8[b], 0, CWQ, C4, b)
            for kt in range(24, 32):
                for b in range(2):
                    mm_p2(kt, b)
            nc.sync.dma_start(out=out_v[:, 0, 512:1024], in_=outs[:, 0, 512:1024])
            nc.scalar.dma_start(out=out_v[:, 1, 512:1024], in_=outs[:, 1, 512:1024])
            for b in range(2):
                epilogue(ps_p2, b * C2, 1024, C2, b)
            nc.sync.dma_start(out=out_v[:, 0, 1024:], in_=outs[:, 0, 1024:])
            nc.scalar.dma_start(out=out_v[:, 1, 1024:], in_=outs[:, 1, 1024:])
    nc.finalize()
    return nc


def _tile128(a):
    """[K, F] -> [128, (K//128)*F] so DRAM layout matches the SBUF tile."""
    k, f = a.shape
    t = k // 128
    return np.ascontiguousarray(
        a.reshape(t, 128, f).transpose(1, 0, 2).reshape(128, t * f)
    )


def _unpack_nibbles(p):
    """packed int8 [N, K//2] -> int [N, K] (low nibble = even k)."""
    u = p.view(np.uint8) if p.dtype == np.int8 else p.astype(np.uint8)
    lo = (u & 15).astype(np.int16)
    hi = ((u >> 4) & 15).astype(np.int16)
    full = np.stack([lo, hi], axis=-1).reshape(p.shape[0], -1)
    return np.where(full > 7, full - 16, full)


_CACHE = {}


def stage_inputs(**inputs):
    x = np.asarray(inputs["x"], dtype=np.float32)
    w16 = np.asarray(inputs["w16"], dtype=np.float32)
    b16 = np.asarray(inputs["b16"], dtype=np.float32)
    q8 = np.asarray(inputs["q8"])
    s8 = np.asarray(inputs["s8"], dtype=np.float32)
    b8 = np.asarray(inputs["b8"], dtype=np.float32)
    p4 = np.asarray(inputs["p4"]).astype(np.int8)
    s4 = np.asarray(inputs["s4"], dtype=np.float32)
    b4 = np.asarray(inputs["b4"], dtype=np.float32)
    p2 = np.asarray(inputs["p2"]).astype(np.int8)
    s2 = np.asarray(inputs["s2"], dtype=np.float32)
    b2 = np.asarray(inputs["b2"], dtype=np.float32)
    idx16 = np.asarray(inputs["idx16"])
    idx8 = np.asarray(inputs["idx8"])
    idx4 = np.asarray(inputs["idx4"])
    idx2 = np.asarray(inputs["idx2"])

    bf16 = ml_dtypes.bfloat16
    f8 = ml_dtypes.float8_e4m3
    permK = np.concatenate([np.arange(0, IN, 2), np.arange(1, IN, 2)])

    # x: [p, kt, blk, tok] flattened; bf16 and fp8 copies
    xTp = np.ascontiguousarray(x.T[permK])  # [4096, 256] f32
    xt4 = xTp.reshape(KT, 128, 2, 128).transpose(1, 0, 2, 3)  # [p, kt, blk, tok]
    xt = np.ascontiguousarray(xt4.reshape(128, KT * M)).astype(bf16)
    x8 = np.ascontiguousarray(xt4.reshape(128, KT * M)).astype(f8)

    q4_full = _unpack_nibbles(p4)  # [N4, 4096] ints
    q2_full = _unpack_nibbles(p2)  # [N2, 4096] ints

    in_maps = []
    for k in range(NCORES):
        w16k = w16[k * C16 : (k + 1) * C16]
        q8k = q8[k * C8 : (k + 1) * C8]
        q4k = q4_full[k * C4 : (k + 1) * C4]
        q2k = q2_full[k * C2 : (k + 1) * C2]
        s8k = s8[k * C8 : (k + 1) * C8, 0]
        s4k = s4[k * C4 : (k + 1) * C4, 0]
        s2k = s2[k * C2 : (k + 1) * C2, 0]
        b16k = b16[k * C16 : (k + 1) * C16]
        b8k = b8[k * C8 : (k + 1) * C8]
        b4k = b4[k * C4 : (k + 1) * C4]
        b2k = b2[k * C2 : (k + 1) * C2]

        w16t = _tile128(np.ascontiguousarray(w16k.T[permK]).astype(bf16))
        q8t = _tile128(np.ascontiguousarray(q8k.astype(np.int8).T[permK]))
        p4u = _tile128(
            np.ascontiguousarray(q4k.astype(np.float32).T[permK]).astype(f8)
        )
        p2u = _tile128(np.ascontiguousarray(q2k.astype(np.int8).T[permK]))

        srow = np.concatenate([np.ones(C16, np.float32), s8k, s4k, s2k])
        sbc = np.ascontiguousarray(
            np.broadcast_to(srow[None, :], (128, NCH))
        ).astype(bf16)
        brow = (
            np.concatenate([b16k, b8k / s8k, b4k / s4k, b2k / s2k])
            .reshape(1, NCH)
            .astype(bf16)
        )

        in_maps.append(
            {"xt": xt, "x8": x8, "w16t": w16t, "q8t": q8t, "p4u": p4u,
             "p2u": p2u, "sbc": sbc, "brow": brow}
        )

    cat_idxs = [
        np.concatenate(
            [
                idx16[k * C16 : (k + 1) * C16],
                idx8[k * C8 : (k + 1) * C8],
                idx4[k * C4 : (k + 1) * C4],
                idx2[k * C2 : (k + 1) * C2],
            ]
        )
        for k in range(NCORES)
    ]
    return in_maps, cat_idxs


def kernel(**inputs):
    in_maps, cat_idxs = stage_inputs(**inputs)
    if "nc" not in _CACHE:
        _enable_ldw_opt()
        _CACHE["nc"] = _build_nc()
    res = run_bass_kernel_spmd(_CACHE["nc"], in_maps, core_ids=list(range(NCORES)))
    _CACHE["last_res"] = res

    out = np.zeros((M, OUT), dtype=np.float32)
    for k in range(NCORES):
        out[:, cat_idxs[k]] = res.results[k]["out"].astype(np.float32)
    return out
